# revision 4
# baseline (speedup 1.0000x reference)
"""CIN (Compressed Interaction Network) kernel for Trainium2, 8 NeuronCores.

Reference computation (per batch b, embedding dim d; x has 40 field vectors):
  h0[s] = relu( sum_{i,j} x_i x_j W0[i,j,s] + b0[s] )          s in 0..128
  nh    = h0[0:64];  d0 = h0[64:128]
  h1[s] = relu( sum_{i<40, j<64} x_i nh_j W1[i,j,s] + b1[s] )
  out   = concat(d0, h1, axis=s) summed over d                 -> (B, 192)

Strategy
--------
Pure data parallel over the batch (B=2048 -> 256 per core).  Per (b, d)
pair ("bd" column, 8192 per core) the two contractions are quadratic /
bilinear forms over small vectors.  Using the polarization identity
  a*b = ((a+b)^2 - a^2 - b^2) / 2
each layer becomes:   project (PE matmul, K-major)  ->  square
(elementwise, PSUM->SBUF)  ->  contract (PE matmul, accumulate).

v2 changes vs baseline:
  * S1L1 projections use PE row-tiling 2-way packing: the 2560 L1 pair
    features are grouped into rectangles (20 x-fields x 32 nh-fields,
    K=52 <= 64), two rectangles run concurrently on PE row-tiles
    (0,0)/(64,0).  20 matmuls in 10 slots.  Rect moving tiles are built
    by SBUF->SBUF DMA (partition shifts) on otherwise-idle DMA engines.
  * Square work split across ACT (direct square), DVE (copy + 2x TT
    square) and GpSimd (DVE copy + mul) with tuned ratios.
  * d0/d1 relu outputs bf16; d-sum reductions moved to GpSimd.
  * All DMA issued from the Sync engine queue (frees GpSimd).
"""

import numpy as np
import ml_dtypes

B, F0, D = 2048, 40, 32
NCORES = 8
BC = B // NCORES       # 256 batches per core
BD = BC * D            # 8192 bd columns per core
NHF = 64               # next-hidden fields (split_half of 128)
S0 = 128               # layer 0 outputs
S1 = 128               # layer 1 outputs
KU = F0 + NHF          # 104

PAIRS0 = [(i, j) for i in range(F0) for j in range(i + 1, F0)]   # 780
R0 = len(PAIRS0) + F0   # 820
NC0 = 7                 # L0 feature chunks (896 padded)
R0P = NC0 * 128
NC1 = 20                # L1 pair-feature blocks (20 x 128 = 2560)
NRECT = 10              # rect blocks (g in 0..1, b in 0..4) x 2 row-tiles

NFREE = 1024            # bd columns per pipeline chunk
NMM = 512               # max free dim per matmul instruction
NCHUNKS = BD // NFREE
NBPC = NFREE // D       # batches per chunk

BF16 = ml_dtypes.bfloat16

# square engine assignment: 'A' = ACT direct square, 'D' = DVE copy + DVE
# 2x tensor-tensor square, 'G' = DVE copy + GpSimd mul.
SQ0_ENG = ['A', 'G', 'A', 'A', 'G', 'A', 'D']                      # 7 L0 blocks
SQ1_ENG = ['A', 'G', 'A', 'G', 'A', 'G', 'A', 'A', 'G', 'A',
           'G', 'A', 'G', 'A', 'A', 'G', 'A', 'D', 'A', 'G']       # 20 L1 blocks

_cached = {}


def _build_host_weights(W0, b0, W1, b1):
    """Precompute projection/coefficient matrices (float64 for the
    cancellation-prone single coefficients, cast to bf16 at the end)."""
    W0 = np.asarray(W0, np.float64)
    W1 = np.asarray(W1, np.float64)

    # ---- layer 0 ----
    p0 = np.zeros((F0, R0P), np.float64)
    c0 = np.zeros((R0P, S0), np.float64)
    Ssym = (W0 + W0.transpose(1, 0, 2)) / 2.0          # [i, j, s]
    for k, (i, j) in enumerate(PAIRS0):
        p0[i, k] = 1.0
        p0[j, k] = 1.0
        c0[k] = Ssym[i, j]
    for i in range(F0):
        k = len(PAIRS0) + i
        p0[i, k] = 1.0
        c0[k] = W0[i, i] - (Ssym[i].sum(axis=0) - Ssym[i, i])
    # chunk layout: feature f lives at partition f%128, free col (f//128)*S0 + s
    c0_sb = c0.reshape(NC0, 128, S0).transpose(1, 0, 2).reshape(128, NC0 * S0)
    p0d = np.zeros((KU, R0P), np.float64)
    p0d[:F0] = p0
    p0d[NHF:KU] = p0          # duplicate rows for PE row-tile packing

    # ---- layer 1: rect layout ----
    # Rect moving tile R_g (g = x-half) layout: partitions 0:32 = nh_lo,
    # 32:52 = x[20g:20g+20]  (row-tile A);  64:96 = nh_hi, 96:116 = x_g
    # (row-tile B).  Rect (g, hh) covers pairs (i in 20g..20g+19,
    # j in 32hh..32hh+31): 640 pairs = 5 blocks of 128.
    # Stationary p1r[128, 10*128]: block (g, b) at cols (g*5+b)*128;
    # rows 0:52 = row-tile A (hh=0), rows 64:116 = row-tile B (hh=1).
    p1r = np.zeros((128, NRECT * 128), np.float64)
    # c1p_sb in vsq emission order: block idx = (g*5+b)*2 + ab
    c1p_sb = np.zeros((128, NC1 * S1), np.float64)
    for g in range(2):
        for b_ in range(5):
            blk = g * 5 + b_
            for t in range(128):
                feat = b_ * 128 + t
                i_loc, j_loc = feat // 32, feat % 32
                i = 20 * g + i_loc
                col = blk * 128 + t
                p1r[j_loc, col] = 1.0
                p1r[32 + i_loc, col] = 1.0
                p1r[64 + j_loc, col] = 1.0
                p1r[96 + i_loc, col] = 1.0
                for ab in range(2):           # ab=0: nh_lo, ab=1: nh_hi
                    j = 32 * ab + j_loc
                    c1p_sb[t, (blk * 2 + ab) * S1:(blk * 2 + ab + 1) * S1] = \
                        W1[i, j] / 2.0
    c1s = np.zeros((KU, S1), np.float64)
    c1s[:NHF] = -0.5 * W1.sum(axis=0)                  # vs nh_j^2
    c1s[NHF:] = -0.5 * W1.sum(axis=1)                  # vs x_i^2

    return {
        "p0": p0d.astype(BF16),
        "c0": c0_sb.astype(BF16),
        "p1r": p1r.astype(BF16),
        "c1p": c1p_sb.astype(BF16),
        "c1s": c1s.astype(BF16),
        "b0": np.asarray(b0, np.float32).reshape(S0, 1),
        "b1": np.asarray(b1, np.float32).reshape(S1, 1),
    }


def _split_multi_waits(nc):
    """The walrus build in this container rejects any instruction carrying
    more than one sync wait ("Too many sync wait commands").  Hoist all but
    one wait of every multi-wait instruction onto same-engine NOPs placed
    immediately before it (engines execute their stream in order, so this
    preserves the happens-before edges)."""
    import concourse.mybir as mybir

    n = 0
    for blk in nc.main_func.blocks:
        insts = blk.instructions
        out = []
        changed = False
        for inst in insts:
            si = getattr(inst, "sync_info", None)
            if si is not None and si.on_wait and len(si.on_wait) > 1:
                waits = list(si.on_wait)
                for w in waits[:-1]:
                    nop = mybir.InstNoOp(
                        name=f"waitsplit_{n}",
                        engine=inst.engine,
                        sync_info=mybir.SyncInfo(on_wait=[w], on_update=[]),
                        bass_nofuse=True,
                    )
                    n += 1
                    out.append(nop)
                si.on_wait = waits[-1:]
                changed = True
            out.append(inst)
        if changed:
            blk.instructions = out
    return n


def _build_nc():
    import concourse.bass as bass
    import concourse.tile as tile
    import concourse.mybir as mybir

    dt = mybir.dt
    AF = mybir.ActivationFunctionType
    ALU = mybir.AluOpType

    nc = bass.Bass()

    NXT = 4                       # x input split into NXT tiles for DMA overlap
    XTW = BD // NXT               # 2048 cols each
    xt_d = [
        nc.dram_tensor(f"xt{t}", [F0, XTW], dt.bfloat16, kind="ExternalInput")
        for t in range(NXT)
    ]
    p0_d = nc.dram_tensor("p0", [KU, R0P], dt.bfloat16, kind="ExternalInput")
    c0_d = nc.dram_tensor("c0", [128, NC0 * S0], dt.bfloat16, kind="ExternalInput")
    p1r_d = nc.dram_tensor("p1r", [128, NRECT * 128], dt.bfloat16,
                           kind="ExternalInput")
    c1p_d = nc.dram_tensor("c1p", [128, NC1 * S1], dt.bfloat16, kind="ExternalInput")
    c1s_d = nc.dram_tensor("c1s", [KU, S1], dt.bfloat16, kind="ExternalInput")
    b0_d = nc.dram_tensor("b0", [S0, 1], dt.float32, kind="ExternalInput")
    b1_d = nc.dram_tensor("b1", [S1, 1], dt.float32, kind="ExternalInput")
    out_d = nc.dram_tensor("out", [S0 - NHF + S1, BC], dt.float32,
                           kind="ExternalOutput")

    with tile.TileContext(nc) as tc:
        with (
            tc.tile_pool(name="const", bufs=1) as const_pool,
            tc.tile_pool(name="xt", bufs=1) as xt_pool,
            tc.tile_pool(name="sq", bufs=12) as sq_pool,
            tc.tile_pool(name="u", bufs=2) as u_pool,
            tc.tile_pool(name="rect", bufs=4) as rect_pool,
            tc.tile_pool(name="d", bufs=2) as d_pool,
            tc.tile_pool(name="outp", bufs=1) as out_pool,
            tc.tile_pool(name="vps", bufs=3, space="PSUM") as vps_pool,
            tc.tile_pool(name="hps", bufs=1, space="PSUM") as h_pool,
        ):
            p0_sb = const_pool.tile([KU, R0P], dt.bfloat16)
            c0_sb = const_pool.tile([128, NC0 * S0], dt.bfloat16)
            p1r_sb = const_pool.tile([128, NRECT * 128], dt.bfloat16)
            c1p_sb = const_pool.tile([128, NC1 * S1], dt.bfloat16)
            c1s_sb = const_pool.tile([KU, S1], dt.bfloat16)
            b0_sb = const_pool.tile([S0, 1], dt.float32)
            b1_sb = const_pool.tile([S1, 1], dt.float32)
            xt_sb = [xt_pool.tile([KU, XTW], dt.bfloat16, tag=f"xt{t}",
                                  name=f"xt_{t}") for t in range(NXT)]
            nc.sync.dma_start(out=xt_sb[0][0:F0, :], in_=xt_d[0][:])
            nc.sync.dma_start(out=xt_sb[0][NHF:KU, :], in_=xt_d[0][:])
            nc.sync.dma_start(out=p0_sb[:], in_=p0_d[:])
            nc.sync.dma_start(out=b0_sb[:], in_=b0_d[:])
            nc.sync.dma_start(out=b1_sb[:], in_=b1_d[:])
            for t in range(1, NXT):
                nc.sync.dma_start(out=xt_sb[t][0:F0, :], in_=xt_d[t][:])
                nc.sync.dma_start(out=xt_sb[t][NHF:KU, :], in_=xt_d[t][:])
            nc.sync.dma_start(out=c0_sb[:], in_=c0_d[:])
            nc.sync.dma_start(out=p1r_sb[:], in_=p1r_d[:])
            nc.sync.dma_start(out=c1p_sb[:], in_=c1p_d[:])
            nc.sync.dma_start(out=c1s_sb[:], in_=c1s_d[:])

            out0_sb = out_pool.tile([S0 - NHF, BC], dt.float32, tag="o0")
            out1_sb = out_pool.tile([S1, BC], dt.float32, tag="o1")

            NH2 = NFREE // NMM   # matmul halves per chunk column block

            def square(eng, dst, src, name):
                if eng == 'A':
                    nc.scalar.square(dst, src)
                else:
                    # DVE copy (PSUM fp32 -> SBUF bf16), then square on SBUF
                    # (DVE 2x tensor-tensor, or GpSimd mul).  DVE cannot read
                    # a PSUM operand twice (NCC_IBVF027).
                    tmp = sq_pool.tile(list(dst.shape), dst.dtype,
                                       tag="sqtmp", bufs=8)
                    nc.vector.tensor_copy(out=tmp[:], in_=src)
                    if eng == 'D':
                        nc.vector.tensor_mul(dst, tmp[:], tmp[:])
                    else:
                        nc.gpsimd.tensor_mul(dst, tmp[:], tmp[:])

            def xap_of(ch):
                xt = xt_sb[(ch * NFREE) // XTW]
                c0_ = (ch * NFREE) % XTW
                return xt[:, c0_:c0_ + NFREE]

            def mm_s1l0(ch, m, vps, row):
                # row 0: PE rows 0:40; row 1: PE rows 64:104 (tile_position
                # packing -- two K=40 matmuls run concurrently)
                lo, hi = (0, F0) if row == 0 else (NHF, KU)
                xa = xap_of(ch)
                for h in range(NH2):
                    hs = slice(h * NMM, (h + 1) * NMM)
                    mi = nc.tensor.matmul(
                        vps[:, hs], p0_sb[lo:hi, m * 128:(m + 1) * 128],
                        xa[lo:hi, hs], start=True, stop=True,
                        tile_position=(lo, 0),
                    )
                    if h > 0:
                        mi.ins.ldweights = False

            def mm_s3l0(ch, m, h0ps, v0sq):
                for h in range(NH2):
                    hs = slice(h * NMM, (h + 1) * NMM)
                    mi = nc.tensor.matmul(
                        h0ps[:, hs], c0_sb[:, m * S0:(m + 1) * S0],
                        v0sq[m][:, hs], start=(m == 0), stop=(m == NC0 - 1),
                    )
                    if h > 0:
                        mi.ins.ldweights = False

            def mm_s1l1_packed(rect, blk, vps, row):
                # row 0: rect row-tile A (partitions 0:52), row 1: row-tile B
                # (partitions 64:116); both use stationary p1r cols blk*128.
                lo, hi = (0, 52) if row == 0 else (64, 116)
                for h in range(NH2):
                    hs = slice(h * NMM, (h + 1) * NMM)
                    mi = nc.tensor.matmul(
                        vps[:, hs], p1r_sb[lo:hi, blk * 128:(blk + 1) * 128],
                        rect[lo:hi, hs], start=True, stop=True,
                        tile_position=(lo, 0),
                    )
                    if h > 0:
                        mi.ins.ldweights = False

            def mm_s3l1(m, h1ps, v1sq, usq):
                for h in range(NH2):
                    hs = slice(h * NMM, (h + 1) * NMM)
                    if m < NC1:
                        mi = nc.tensor.matmul(
                            h1ps[:, hs], c1p_sb[:, m * S1:(m + 1) * S1],
                            v1sq[m][:, hs], start=(m == 0), stop=False,
                        )
                    else:
                        mi = nc.tensor.matmul(
                            h1ps[:, hs], c1s_sb[:], usq[:, hs],
                            start=False, stop=True,
                        )
                    if h > 0:
                        mi.ins.ldweights = False

            # Per-chunk live state, keyed by chunk index.
            st = {}

            for i in range(NCHUNKS + 2):
                cA = i          # S1L0 of chunk i
                cB = i - 1      # S3L0 + post0 + S1L1 of chunk i-1
                cE = i - 2      # S3L1 + post1 of chunk i-2

                if cA < NCHUNKS:
                    st[cA] = {"v0sq": [], "v1sq": []}

                # ---- step 1: S3L0(cB) interleaved with S1L0(cA) ----
                for m in range(NC0):
                    if cA < NCHUNKS and m % 2 == 0:
                        # emit a packed pair (m, m+1) of K=40 projections
                        for mm in (m, m + 1):
                            if mm >= NC0:
                                break
                            vps = vps_pool.tile([128, NFREE], dt.float32,
                                                tag="vps", name=f"vps0_{cA}_{mm}")
                            mm_s1l0(cA, mm, vps, mm - m)
                            vsq = sq_pool.tile([128, NFREE], dt.bfloat16,
                                               tag="v0sq", bufs=16,
                                               name=f"v0sq_{cA}_{mm}")
                            square(SQ0_ENG[mm], vsq[:], vps[:],
                                   f"sq0_{cA}_{mm}")
                            st[cA]["v0sq"].append(vsq)
                    if 0 <= cB < NCHUNKS:
                        if m == 0:
                            st[cB]["h0ps"] = h_pool.tile(
                                [S0, NFREE], dt.float32, tag="hps",
                                name=f"h0ps_{cB}")
                        mm_s3l0(cB, m, st[cB]["h0ps"], st[cB]["v0sq"])

                # ---- step 2: post0(cB): relu->u, usq, rect DMA, d0 ----
                if 0 <= cB < NCHUNKS:
                    h0ps = st[cB]["h0ps"]
                    xa = xap_of(cB)
                    u = u_pool.tile([NHF, NFREE], dt.bfloat16, tag="u")
                    nc.scalar.activation(u[:], h0ps[0:NHF, :], AF.Relu,
                                         bias=b0_sb[0:NHF, 0:1], scale=1.0)
                    # rect tiles for S1L1: [nh_lo; x_g | nh_hi; x_g]
                    rects = []
                    for g in range(2):
                        rg = rect_pool.tile([128, NFREE], dt.bfloat16,
                                            tag=f"rect{g}",
                                            name=f"rect{g}_{cB}")
                        xg = xa[20 * g:20 * g + 20, :]
                        nc.sync.dma_start(out=rg[32:52, :], in_=xg)
                        nc.sync.dma_start(out=rg[96:116, :], in_=xg)
                        nc.sync.dma_start(out=rg[0:32, :], in_=u[0:32, :])
                        nc.sync.dma_start(out=rg[64:96, :], in_=u[32:64, :])
                        rects.append(rg)
                    st[cB]["rects"] = rects
                    # usq: rows 0:64 = nh^2 (from u), 64:104 = x^2 (from the
                    # duplicated x rows of xt at partitions 64:104)
                    usq = u_pool.tile([KU, NFREE], dt.bfloat16, tag="usq")
                    nc.vector.tensor_mul(usq[0:NHF, :], u[:], u[:])
                    nc.vector.tensor_mul(usq[NHF:KU, :], xa[NHF:KU, :],
                                         xa[NHF:KU, :])
                    st[cB]["usq"] = usq
                    d0 = d_pool.tile([S0 - NHF, NBPC, D], dt.bfloat16, tag="d0")
                    nc.scalar.activation(d0[:], h0ps[NHF:S0, :], AF.Relu,
                                         bias=b0_sb[NHF:S0, 0:1], scale=1.0)
                    nc.vector.tensor_reduce(
                        out=out0_sb[:, cB * NBPC:(cB + 1) * NBPC],
                        in_=d0[:], axis=mybir.AxisListType.X, op=ALU.add,
                    )

                # ---- step 3: S3L1(cE) interleaved with packed S1L1(cB) ----
                e_list = list(range(NC1 + 1)) if 0 <= cE else []
                pair_list = (
                    [(g, b_) for g in range(2) for b_ in range(5)]
                    if 0 <= cB < NCHUNKS else []
                )
                ei = 0
                if e_list:
                    st[cE]["h1ps"] = h_pool.tile([S1, NFREE], dt.float32,
                                                 tag="hps",
                                                 name=f"h1ps_{cE}")
                    for _ in range(2):
                        if ei < len(e_list):
                            mm_s3l1(e_list[ei], st[cE]["h1ps"],
                                    st[cE]["v1sq"], st[cE]["usq"])
                            ei += 1
                for g, b_ in pair_list:
                    blk = g * 5 + b_
                    rg = st[cB]["rects"][g]
                    for ab in range(2):
                        vps = vps_pool.tile([128, NFREE], dt.float32,
                                            tag="vps",
                                            name=f"vps1_{cB}_{blk}_{ab}")
                        mm_s1l1_packed(rg, blk, vps, ab)
                        vsq = sq_pool.tile([128, NFREE], dt.bfloat16,
                                           tag="v1sq", bufs=24,
                                           name=f"v1sq_{cB}_{blk}_{ab}")
                        square(SQ1_ENG[blk * 2 + ab], vsq[:], vps[:],
                               f"sq1_{cB}_{blk}_{ab}")
                        st[cB]["v1sq"].append(vsq)
                    for _ in range(2):
                        if e_list and ei < len(e_list):
                            mm_s3l1(e_list[ei], st[cE]["h1ps"],
                                    st[cE]["v1sq"], st[cE]["usq"])
                            ei += 1
                while e_list and ei < len(e_list):
                    mm_s3l1(e_list[ei], st[cE]["h1ps"],
                            st[cE]["v1sq"], st[cE]["usq"])
                    ei += 1

                # ---- step 4: post1(cE): relu d1 + reduce ----
                if 0 <= cE:
                    d1 = d_pool.tile([S1, NBPC, D], dt.bfloat16, tag="d1")
                    nc.scalar.activation(d1[:], st[cE]["h1ps"][:], AF.Relu,
                                         bias=b1_sb[:, 0:1], scale=1.0)
                    nc.vector.tensor_reduce(
                        out=out1_sb[:, cE * NBPC:(cE + 1) * NBPC],
                        in_=d1[:], axis=mybir.AxisListType.X, op=ALU.add,
                    )
                    del st[cE]

            nc.sync.dma_start(out=out_d[0:S0 - NHF, :], in_=out0_sb[:])
            nc.sync.dma_start(out=out_d[S0 - NHF:, :], in_=out1_sb[:])

    _split_multi_waits(nc)
    return nc


def kernel(x, W0, b0, W1, b1):
    from concourse.bass_utils import run_bass_kernel_spmd

    x = np.asarray(x)
    w = _build_host_weights(W0, b0, W1, b1)

    if "nc" not in _cached:
        _cached["nc"] = _build_nc()
    nc = _cached["nc"]

    NXT = 4
    XTW = BD // NXT
    in_maps = []
    for c in range(NCORES):
        xs = x[c * BC:(c + 1) * BC]                        # [256, 40, 32]
        xtc = np.ascontiguousarray(
            xs.transpose(1, 0, 2).reshape(F0, BD)
        ).astype(BF16)                                     # [40, 8192]
        m = {f"xt{t}": np.ascontiguousarray(xtc[:, t * XTW:(t + 1) * XTW])
             for t in range(NXT)}
        m.update(w)
        in_maps.append(m)

    import os
    trace = bool(os.environ.get("CIN_TRACE"))
    res = run_bass_kernel_spmd(nc, in_maps, list(range(NCORES)), trace=trace)
    _cached["last_res"] = res
    outs = []
    for c in range(NCORES):
        o = res.results[c]["out"]                          # [192, 256]
        outs.append(np.ascontiguousarray(o.T))             # [256, 192]
    return np.concatenate(outs, axis=0).astype(np.float32)


# revision 5
# speedup vs baseline: 1.1984x; 1.1984x over previous
"""CIN (Compressed Interaction Network) kernel for Trainium2, 8 NeuronCores.

Reference computation (per batch b, embedding dim d; x has 40 field vectors):
  h0[s] = relu( sum_{i,j} x_i x_j W0[i,j,s] + b0[s] )          s in 0..128
  nh    = h0[0:64];  d0 = h0[64:128]
  h1[s] = relu( sum_{i<40, j<64} x_i nh_j W1[i,j,s] + b1[s] )
  out   = concat(d0, h1, axis=s) summed over d                 -> (B, 192)

Strategy
--------
Pure data parallel over the batch (B=2048 -> 256 per core).  Per (b, d)
pair ("bd" column, 8192 per core) the two contractions are quadratic /
bilinear forms over small vectors.  Using the polarization identity
  a*b = ((a+b)^2 - a^2 - b^2) / 2
each layer becomes:   project (PE matmul, K-major)  ->  square
(elementwise, PSUM->SBUF)  ->  contract (PE matmul, accumulate).

v2 changes vs baseline:
  * S1L1 projections use PE row-tiling 2-way packing: the 2560 L1 pair
    features are grouped into rectangles (20 x-fields x 32 nh-fields,
    K=52 <= 64), two rectangles run concurrently on PE row-tiles
    (0,0)/(64,0).  20 matmuls in 10 slots.  Rect moving tiles are built
    by SBUF->SBUF DMA (partition shifts) on otherwise-idle DMA engines.
  * Square work split across ACT (direct square), DVE (copy + 2x TT
    square) and GpSimd (DVE copy + mul) with tuned ratios.
  * d0/d1 relu outputs bf16; d-sum reductions moved to GpSimd.
  * All DMA issued from the Sync engine queue (frees GpSimd).
"""

import numpy as np
import ml_dtypes

B, F0, D = 2048, 40, 32
NCORES = 8
BC = B // NCORES       # 256 batches per core
BD = BC * D            # 8192 bd columns per core
NHF = 64               # next-hidden fields (split_half of 128)
S0 = 128               # layer 0 outputs
S1 = 128               # layer 1 outputs
KU = F0 + NHF          # 104

PAIRS0 = [(i, j) for i in range(F0) for j in range(i + 1, F0)]   # 780
R0 = len(PAIRS0) + F0   # 820
NC0 = 7                 # L0 feature chunks (896 padded)
R0P = NC0 * 128
NC1 = 20                # L1 pair-feature blocks (20 x 128 = 2560)
NRECT = 10              # rect blocks (g in 0..1, b in 0..4) x 2 row-tiles

NFREE = 1024            # bd columns per pipeline chunk
NMM = 512               # max free dim per matmul instruction
NCHUNKS = BD // NFREE
NBPC = NFREE // D       # batches per chunk

BF16 = ml_dtypes.bfloat16

# square engine assignment: 'A' = ACT direct square, 'D' = DVE copy + DVE
# 2x tensor-tensor square, 'G' = DVE copy + GpSimd mul.
SQ0_ENG = ['A', 'G', 'A', 'G', 'A', 'G', 'A']                      # 7 L0 blocks
SQ1_ENG = ['A', 'G', 'A', 'A', 'G', 'A', 'G', 'A', 'A', 'G',
           'A', 'G', 'A', 'A', 'G', 'A', 'G', 'A', 'A', 'G']       # 20 L1 blocks

_cached = {}


def _build_host_weights(W0, b0, W1, b1):
    """Precompute projection/coefficient matrices (float64 for the
    cancellation-prone single coefficients, cast to bf16 at the end)."""
    W0 = np.asarray(W0, np.float64)
    W1 = np.asarray(W1, np.float64)

    # ---- layer 0 ----
    p0 = np.zeros((F0, R0P), np.float64)
    c0 = np.zeros((R0P, S0), np.float64)
    Ssym = (W0 + W0.transpose(1, 0, 2)) / 2.0          # [i, j, s]
    for k, (i, j) in enumerate(PAIRS0):
        p0[i, k] = 1.0
        p0[j, k] = 1.0
        c0[k] = Ssym[i, j]
    for i in range(F0):
        k = len(PAIRS0) + i
        p0[i, k] = 1.0
        c0[k] = W0[i, i] - (Ssym[i].sum(axis=0) - Ssym[i, i])
    # chunk layout: feature f lives at partition f%128, free col (f//128)*S0 + s
    c0_sb = c0.reshape(NC0, 128, S0).transpose(1, 0, 2).reshape(128, NC0 * S0)
    p0d = np.zeros((KU, R0P), np.float64)
    p0d[:F0] = p0
    p0d[NHF:KU] = p0          # duplicate rows for PE row-tile packing

    # ---- layer 1 ----
    # U layout (SBUF partition rows must start 32-aligned): nh_j at row j
    # (0:64), x_i at row 64+i (64:104).
    R1 = F0 * NHF
    p1 = np.zeros((KU, R1), np.float64)
    c1p = np.zeros((R1, S1), np.float64)
    for i in range(F0):
        for j in range(NHF):
            k = i * NHF + j
            p1[NHF + i, k] = 1.0
            p1[j, k] = 1.0
            c1p[k] = W1[i, j] / 2.0
    c1p_sb = c1p.reshape(NC1, 128, S1).transpose(1, 0, 2).reshape(128, NC1 * S1)
    c1s = np.zeros((KU, S1), np.float64)
    c1s[:NHF] = -0.5 * W1.sum(axis=0)                  # vs nh_j^2
    c1s[NHF:] = -0.5 * W1.sum(axis=1)                  # vs x_i^2

    return {
        "p0": p0d.astype(BF16),
        "c0": c0_sb.astype(BF16),
        "p1": p1.astype(BF16),
        "c1p": c1p_sb.astype(BF16),
        "c1s": c1s.astype(BF16),
        "b0": np.asarray(b0, np.float32).reshape(S0, 1),
        "b1": np.asarray(b1, np.float32).reshape(S1, 1),
    }


def _split_multi_waits(nc):
    """The walrus build in this container rejects any instruction carrying
    more than one sync wait ("Too many sync wait commands").  Hoist all but
    one wait of every multi-wait instruction onto same-engine NOPs placed
    immediately before it (engines execute their stream in order, so this
    preserves the happens-before edges)."""
    import concourse.mybir as mybir

    n = 0
    for blk in nc.main_func.blocks:
        insts = blk.instructions
        out = []
        changed = False
        for inst in insts:
            si = getattr(inst, "sync_info", None)
            if si is not None and si.on_wait and len(si.on_wait) > 1:
                waits = list(si.on_wait)
                for w in waits[:-1]:
                    nop = mybir.InstNoOp(
                        name=f"waitsplit_{n}",
                        engine=inst.engine,
                        sync_info=mybir.SyncInfo(on_wait=[w], on_update=[]),
                        bass_nofuse=True,
                    )
                    n += 1
                    out.append(nop)
                si.on_wait = waits[-1:]
                changed = True
            out.append(inst)
        if changed:
            blk.instructions = out
    return n


def _build_nc():
    import concourse.bass as bass
    import concourse.tile as tile
    import concourse.mybir as mybir

    dt = mybir.dt
    AF = mybir.ActivationFunctionType
    ALU = mybir.AluOpType

    nc = bass.Bass()

    NXT = 4                       # x input split into NXT tiles for DMA overlap
    XTW = BD // NXT               # 2048 cols each
    xt_d = [
        nc.dram_tensor(f"xt{t}", [F0, XTW], dt.bfloat16, kind="ExternalInput")
        for t in range(NXT)
    ]
    p0_d = nc.dram_tensor("p0", [KU, R0P], dt.bfloat16, kind="ExternalInput")
    c0_d = nc.dram_tensor("c0", [128, NC0 * S0], dt.bfloat16, kind="ExternalInput")
    p1_d = nc.dram_tensor("p1", [KU, F0 * NHF], dt.bfloat16,
                          kind="ExternalInput")
    c1p_d = nc.dram_tensor("c1p", [128, NC1 * S1], dt.bfloat16, kind="ExternalInput")
    c1s_d = nc.dram_tensor("c1s", [KU, S1], dt.bfloat16, kind="ExternalInput")
    b0_d = nc.dram_tensor("b0", [S0, 1], dt.float32, kind="ExternalInput")
    b1_d = nc.dram_tensor("b1", [S1, 1], dt.float32, kind="ExternalInput")
    out_d = nc.dram_tensor("out", [S0 - NHF + S1, BC], dt.float32,
                           kind="ExternalOutput")

    with tile.TileContext(nc) as tc:
        with (
            tc.tile_pool(name="const", bufs=1) as const_pool,
            tc.tile_pool(name="xt", bufs=1) as xt_pool,
            tc.tile_pool(name="sq", bufs=12) as sq_pool,
            tc.tile_pool(name="u", bufs=2) as u_pool,
            tc.tile_pool(name="d", bufs=2) as d_pool,
            tc.tile_pool(name="outp", bufs=1) as out_pool,
            tc.tile_pool(name="vps", bufs=3, space="PSUM") as vps_pool,
            tc.tile_pool(name="hps", bufs=1, space="PSUM") as h_pool,
        ):
            p0_sb = const_pool.tile([KU, R0P], dt.bfloat16)
            c0_sb = const_pool.tile([128, NC0 * S0], dt.bfloat16)
            p1_sb = const_pool.tile([KU, F0 * NHF], dt.bfloat16)
            c1p_sb = const_pool.tile([128, NC1 * S1], dt.bfloat16)
            c1s_sb = const_pool.tile([KU, S1], dt.bfloat16)
            b0_sb = const_pool.tile([S0, 1], dt.float32)
            b1_sb = const_pool.tile([S1, 1], dt.float32)
            xt_sb = [xt_pool.tile([KU, XTW], dt.bfloat16, tag=f"xt{t}",
                                  name=f"xt_{t}") for t in range(NXT)]
            nc.sync.dma_start(out=xt_sb[0][0:F0, :], in_=xt_d[0][:])
            nc.sync.dma_start(out=xt_sb[0][NHF:KU, :], in_=xt_d[0][:])
            nc.sync.dma_start(out=p0_sb[:], in_=p0_d[:])
            nc.sync.dma_start(out=b0_sb[:], in_=b0_d[:])
            nc.sync.dma_start(out=b1_sb[:], in_=b1_d[:])
            for t in range(1, NXT):
                nc.sync.dma_start(out=xt_sb[t][0:F0, :], in_=xt_d[t][:])
                nc.sync.dma_start(out=xt_sb[t][NHF:KU, :], in_=xt_d[t][:])
            nc.sync.dma_start(out=c0_sb[:], in_=c0_d[:])
            nc.sync.dma_start(out=p1_sb[:], in_=p1_d[:])
            nc.sync.dma_start(out=c1p_sb[:], in_=c1p_d[:])
            nc.sync.dma_start(out=c1s_sb[:], in_=c1s_d[:])

            out0_sb = out_pool.tile([S0 - NHF, BC], dt.float32, tag="o0")
            out1_sb = out_pool.tile([S1, BC], dt.float32, tag="o1")

            NH2 = NFREE // NMM   # matmul halves per chunk column block

            def square(eng, dst, src, name):
                if eng == 'A':
                    nc.scalar.square(dst, src)
                else:
                    # DVE copy (PSUM fp32 -> SBUF bf16), then square on SBUF
                    # (DVE 2x tensor-tensor, or GpSimd mul).  DVE cannot read
                    # a PSUM operand twice (NCC_IBVF027).
                    tmp = sq_pool.tile(list(dst.shape), dst.dtype,
                                       tag="sqtmp", bufs=8)
                    nc.vector.tensor_copy(out=tmp[:], in_=src)
                    if eng == 'D':
                        nc.vector.tensor_mul(dst, tmp[:], tmp[:])
                    else:
                        nc.gpsimd.tensor_mul(dst, tmp[:], tmp[:])

            def xap_of(ch):
                xt = xt_sb[(ch * NFREE) // XTW]
                c0_ = (ch * NFREE) % XTW
                return xt[:, c0_:c0_ + NFREE]

            def mm_s1l0(ch, m, vps, row):
                # row 0: PE rows 0:40; row 1: PE rows 64:104 (tile_position
                # packing -- two K=40 matmuls run concurrently)
                lo, hi = (0, F0) if row == 0 else (NHF, KU)
                xa = xap_of(ch)
                for h in range(NH2):
                    hs = slice(h * NMM, (h + 1) * NMM)
                    mi = nc.tensor.matmul(
                        vps[:, hs], p0_sb[lo:hi, m * 128:(m + 1) * 128],
                        xa[lo:hi, hs], start=True, stop=True,
                        tile_position=(lo, 0),
                    )
                    if h > 0:
                        mi.ins.ldweights = False

            def mm_s3l0(ch, m, h0ps, v0sq):
                for h in range(NH2):
                    hs = slice(h * NMM, (h + 1) * NMM)
                    mi = nc.tensor.matmul(
                        h0ps[:, hs], c0_sb[:, m * S0:(m + 1) * S0],
                        v0sq[m][:, hs], start=(m == 0), stop=(m == NC0 - 1),
                    )
                    if h > 0:
                        mi.ins.ldweights = False

            def mm_s1l1(m, vps, u):
                for h in range(NH2):
                    hs = slice(h * NMM, (h + 1) * NMM)
                    mi = nc.tensor.matmul(
                        vps[:, hs], p1_sb[:, m * 128:(m + 1) * 128],
                        u[:, hs], start=True, stop=True,
                    )
                    if h > 0:
                        mi.ins.ldweights = False

            def mm_s3l1(m, h1ps, v1sq, usq):
                for h in range(NH2):
                    hs = slice(h * NMM, (h + 1) * NMM)
                    if m < NC1:
                        mi = nc.tensor.matmul(
                            h1ps[:, hs], c1p_sb[:, m * S1:(m + 1) * S1],
                            v1sq[m][:, hs], start=(m == 0), stop=False,
                        )
                    else:
                        mi = nc.tensor.matmul(
                            h1ps[:, hs], c1s_sb[:], usq[:, hs],
                            start=False, stop=True,
                        )
                    if h > 0:
                        mi.ins.ldweights = False

            # Per-chunk live state, keyed by chunk index.
            st = {}

            for i in range(NCHUNKS + 2):
                cA = i          # S1L0 of chunk i
                cB = i - 1      # S3L0 + post0 + S1L1 of chunk i-1
                cE = i - 2      # S3L1 + post1 of chunk i-2

                if cA < NCHUNKS:
                    st[cA] = {"v0sq": [], "v1sq": []}

                # ---- step 1: S3L0(cB) interleaved with S1L0(cA) ----
                for m in range(NC0):
                    if cA < NCHUNKS and m % 2 == 0:
                        # emit a packed pair (m, m+1) of K=40 projections
                        for mm in (m, m + 1):
                            if mm >= NC0:
                                break
                            vps = vps_pool.tile([128, NFREE], dt.float32,
                                                tag="vps", name=f"vps0_{cA}_{mm}")
                            mm_s1l0(cA, mm, vps, mm - m)
                            vsq = sq_pool.tile([128, NFREE], dt.bfloat16,
                                               tag="v0sq", bufs=16,
                                               name=f"v0sq_{cA}_{mm}")
                            square(SQ0_ENG[mm], vsq[:], vps[:],
                                   f"sq0_{cA}_{mm}")
                            st[cA]["v0sq"].append(vsq)
                    if 0 <= cB < NCHUNKS:
                        if m == 0:
                            st[cB]["h0ps"] = h_pool.tile(
                                [S0, NFREE], dt.float32, tag="hps",
                                name=f"h0ps_{cB}")
                        mm_s3l0(cB, m, st[cB]["h0ps"], st[cB]["v0sq"])

                # ---- step 2: post0(cB): relu->u, usq, rect DMA, d0 ----
                if 0 <= cB < NCHUNKS:
                    h0ps = st[cB]["h0ps"]
                    xa = xap_of(cB)
                    u = u_pool.tile([KU, NFREE], dt.bfloat16, tag="u")
                    nc.vector.tensor_copy(out=u[NHF:KU, :], in_=xa[NHF:KU, :])
                    nc.scalar.activation(u[0:NHF, :], h0ps[0:NHF, :], AF.Relu,
                                         bias=b0_sb[0:NHF, 0:1], scale=1.0)
                    usq = u_pool.tile([KU, NFREE], dt.bfloat16, tag="usq")
                    nc.vector.tensor_mul(usq[:], u[:], u[:])
                    st[cB]["u"] = u
                    st[cB]["usq"] = usq
                    d0 = d_pool.tile([S0 - NHF, NBPC, D], dt.bfloat16, tag="d0")
                    nc.scalar.activation(d0[:], h0ps[NHF:S0, :], AF.Relu,
                                         bias=b0_sb[NHF:S0, 0:1], scale=1.0)
                    nc.vector.tensor_reduce(
                        out=out0_sb[:, cB * NBPC:(cB + 1) * NBPC],
                        in_=d0[:], axis=mybir.AxisListType.X, op=ALU.add,
                    )

                # ---- step 3: S3L1(cE) interleaved with packed S1L1(cB) ----
                e_list = list(range(NC1 + 1)) if 0 <= cE else []
                d_list = list(range(NC1)) if 0 <= cB < NCHUNKS else []
                ei = 0
                if e_list:
                    st[cE]["h1ps"] = h_pool.tile([S1, NFREE], dt.float32,
                                                 tag="hps",
                                                 name=f"h1ps_{cE}")
                    for _ in range(2):
                        if ei < len(e_list):
                            mm_s3l1(e_list[ei], st[cE]["h1ps"],
                                    st[cE]["v1sq"], st[cE]["usq"])
                            ei += 1
                for m in d_list:
                    vps = vps_pool.tile([128, NFREE], dt.float32, tag="vps",
                                        name=f"vps1_{cB}_{m}")
                    mm_s1l1(m, vps, st[cB]["u"])
                    vsq = sq_pool.tile([128, NFREE], dt.bfloat16,
                                       tag="v1sq", bufs=24,
                                       name=f"v1sq_{cB}_{m}")
                    square(SQ1_ENG[m], vsq[:], vps[:], f"sq1_{cB}_{m}")
                    st[cB]["v1sq"].append(vsq)
                    if e_list and ei < len(e_list):
                        mm_s3l1(e_list[ei], st[cE]["h1ps"],
                                st[cE]["v1sq"], st[cE]["usq"])
                        ei += 1
                while e_list and ei < len(e_list):
                    mm_s3l1(e_list[ei], st[cE]["h1ps"],
                            st[cE]["v1sq"], st[cE]["usq"])
                    ei += 1

                # ---- step 4: post1(cE): relu d1 + reduce ----
                if 0 <= cE:
                    d1 = d_pool.tile([S1, NBPC, D], dt.bfloat16, tag="d1")
                    nc.scalar.activation(d1[:], st[cE]["h1ps"][:], AF.Relu,
                                         bias=b1_sb[:, 0:1], scale=1.0)
                    nc.vector.tensor_reduce(
                        out=out1_sb[:, cE * NBPC:(cE + 1) * NBPC],
                        in_=d1[:], axis=mybir.AxisListType.X, op=ALU.add,
                    )
                    del st[cE]

            nc.sync.dma_start(out=out_d[0:S0 - NHF, :], in_=out0_sb[:])
            nc.sync.dma_start(out=out_d[S0 - NHF:, :], in_=out1_sb[:])

    _split_multi_waits(nc)
    return nc


def kernel(x, W0, b0, W1, b1):
    from concourse.bass_utils import run_bass_kernel_spmd

    x = np.asarray(x)
    w = _build_host_weights(W0, b0, W1, b1)

    if "nc" not in _cached:
        _cached["nc"] = _build_nc()
    nc = _cached["nc"]

    NXT = 4
    XTW = BD // NXT
    in_maps = []
    for c in range(NCORES):
        xs = x[c * BC:(c + 1) * BC]                        # [256, 40, 32]
        xtc = np.ascontiguousarray(
            xs.transpose(1, 0, 2).reshape(F0, BD)
        ).astype(BF16)                                     # [40, 8192]
        m = {f"xt{t}": np.ascontiguousarray(xtc[:, t * XTW:(t + 1) * XTW])
             for t in range(NXT)}
        m.update(w)
        in_maps.append(m)

    import os
    trace = bool(os.environ.get("CIN_TRACE"))
    res = run_bass_kernel_spmd(nc, in_maps, list(range(NCORES)), trace=trace)
    _cached["last_res"] = res
    outs = []
    for c in range(NCORES):
        o = res.results[c]["out"]                          # [192, 256]
        outs.append(np.ascontiguousarray(o.T))             # [256, 192]
    return np.concatenate(outs, axis=0).astype(np.float32)


# revision 6
# speedup vs baseline: 1.4152x; 1.1809x over previous
"""CIN (Compressed Interaction Network) kernel for Trainium2, 8 NeuronCores.

Reference computation (per batch b, embedding dim d; x has 40 field vectors):
  h0[s] = relu( sum_{i,j} x_i x_j W0[i,j,s] + b0[s] )          s in 0..128
  nh    = h0[0:64];  d0 = h0[64:128]
  h1[s] = relu( sum_{i<40, j<64} x_i nh_j W1[i,j,s] + b1[s] )
  out   = concat(d0, h1, axis=s) summed over d                 -> (B, 192)

Strategy
--------
Pure data parallel over the batch (B=2048 -> 256 per core).  Per (b, d)
pair ("bd" column, 8192 per core) the two contractions are quadratic /
bilinear forms over small vectors.  Using the polarization identity
  a*b = ((a+b)^2 - a^2 - b^2) / 2
each layer becomes:   project (PE matmul, K-major)  ->  square
(ACT/DVE elementwise, PSUM->SBUF)  ->  contract (PE matmul, accumulate).
This avoids all transposes and partition broadcasts; every matmul is
K-major with bd on the free axis.  All matmul operands are bf16 (fp32
accumulation in PSUM); coefficient matrices are precomputed on host.

Layer 0: features = 780 pair sums (x_i+x_j, i<j) + 40 singles (x_i),
padded to 896 = 7*128.  Layer 1: 2560 pair sums (x_i + nh_j) in 20
chunks + one K=104 contraction against squared [x; nh] singles.
"""

import numpy as np
import ml_dtypes

B, F0, D = 2048, 40, 32
NCORES = 8
BC = B // NCORES       # 256 batches per core
BD = BC * D            # 8192 bd columns per core
NHF = 64               # next-hidden fields (split_half of 128)
S0 = 128               # layer 0 outputs
S1 = 128               # layer 1 outputs
KU = F0 + NHF          # 104

PAIRS0 = [(i, j) for i in range(F0) for j in range(i + 1, F0)]   # 780
R0 = len(PAIRS0) + F0   # 820
NC0 = 7                 # L0 feature chunks (896 padded)
R0P = NC0 * 128
R1 = F0 * NHF           # 2560 L1 pair features
NC1 = R1 // 128         # 20

NFREE = 1024            # bd columns per pipeline chunk
NMM = 512               # max free dim per matmul instruction
NCHUNKS = BD // NFREE
NBPC = NFREE // D       # batches per chunk

BF16 = ml_dtypes.bfloat16

_cached = {}


def _build_host_weights(W0, b0, W1, b1):
    """Precompute projection/coefficient matrices (float64 for the
    cancellation-prone single coefficients, cast to bf16 at the end)."""
    W0 = np.asarray(W0, np.float64)
    W1 = np.asarray(W1, np.float64)

    # ---- layer 0 ----
    p0 = np.zeros((F0, R0P), np.float64)
    c0 = np.zeros((R0P, S0), np.float64)
    Ssym = (W0 + W0.transpose(1, 0, 2)) / 2.0          # [i, j, s]
    for k, (i, j) in enumerate(PAIRS0):
        p0[i, k] = 1.0
        p0[j, k] = 1.0
        c0[k] = Ssym[i, j]
    for i in range(F0):
        k = len(PAIRS0) + i
        p0[i, k] = 1.0
        c0[k] = W0[i, i] - (Ssym[i].sum(axis=0) - Ssym[i, i])
    # chunk layout: feature f lives at partition f%128, free col (f//128)*S0 + s
    c0_sb = c0.reshape(NC0, 128, S0).transpose(1, 0, 2).reshape(128, NC0 * S0)

    # ---- layer 1 ----
    # U layout (SBUF partition rows must start 32-aligned): nh_j at row j
    # (0:64), x_i at row 64+i (64:104).
    p1 = np.zeros((KU, R1), np.float64)
    c1p = np.zeros((R1, S1), np.float64)
    for i in range(F0):
        for j in range(NHF):
            k = i * NHF + j
            p1[NHF + i, k] = 1.0
            p1[j, k] = 1.0
            c1p[k] = W1[i, j] / 2.0
    c1p_sb = c1p.reshape(NC1, 128, S1).transpose(1, 0, 2).reshape(128, NC1 * S1)
    c1s = np.zeros((KU, S1), np.float64)
    c1s[:NHF] = -0.5 * W1.sum(axis=0)                  # vs nh_j^2
    c1s[NHF:] = -0.5 * W1.sum(axis=1)                  # vs x_i^2

    p0d = np.zeros((KU, R0P), np.float64)
    p0d[:F0] = p0
    p0d[NHF:KU] = p0
    return {
        "p0": p0d.astype(BF16),
        "c0": c0_sb.astype(BF16),
        "p1": p1.astype(BF16),
        "c1p": c1p_sb.astype(BF16),
        "c1s": c1s.astype(BF16),
        "b0": np.asarray(b0, np.float32).reshape(S0, 1),
        "b1": np.asarray(b1, np.float32).reshape(S1, 1),
    }


def _split_multi_waits(nc):
    """The walrus build in this container rejects any instruction carrying
    more than one sync wait ("Too many sync wait commands").  Hoist all but
    one wait of every multi-wait instruction onto same-engine NOPs placed
    immediately before it (engines execute their stream in order, so this
    preserves the happens-before edges)."""
    import concourse.mybir as mybir

    n = 0
    for blk in nc.main_func.blocks:
        insts = blk.instructions
        out = []
        changed = False
        for inst in insts:
            si = getattr(inst, "sync_info", None)
            if si is not None and si.on_wait and len(si.on_wait) > 1:
                waits = list(si.on_wait)
                for w in waits[:-1]:
                    nop = mybir.InstNoOp(
                        name=f"waitsplit_{n}",
                        engine=inst.engine,
                        sync_info=mybir.SyncInfo(on_wait=[w], on_update=[]),
                        bass_nofuse=True,
                    )
                    n += 1
                    out.append(nop)
                si.on_wait = waits[-1:]
                changed = True
            out.append(inst)
        if changed:
            blk.instructions = out
    return n


def _patch_ldw_opt():
    """Re-enable walrus's LDWEIGHTS dedupe (consecutive matmuls that reuse
    the same stationary operand skip the reload).  The concourse wrapper
    hardcodes --enable-ldw-opt=false."""
    # Tried flipping --enable-ldw-opt=true: walrus rejects bass-emitted
    # LDWEIGHTS under that pass ("not compatible with LDW optimization").
    return


def _build_nc():
    import concourse.bass as bass
    import concourse.tile as tile
    import concourse.mybir as mybir

    dt = mybir.dt
    AF = mybir.ActivationFunctionType
    ALU = mybir.AluOpType

    nc = bass.Bass()

    NXT = 4                       # x input split into NXT tiles for DMA overlap
    XTW = BD // NXT               # 2048 cols each
    xt_d = [
        nc.dram_tensor(f"xt{t}", [F0, XTW], dt.bfloat16, kind="ExternalInput")
        for t in range(NXT)
    ]
    p0_d = nc.dram_tensor("p0", [KU, R0P], dt.bfloat16, kind="ExternalInput")
    c0_d = nc.dram_tensor("c0", [128, NC0 * S0], dt.bfloat16, kind="ExternalInput")
    p1_d = nc.dram_tensor("p1", [KU, R1], dt.bfloat16, kind="ExternalInput")
    c1p_d = nc.dram_tensor("c1p", [128, NC1 * S1], dt.bfloat16, kind="ExternalInput")
    c1s_d = nc.dram_tensor("c1s", [KU, S1], dt.bfloat16, kind="ExternalInput")
    b0_d = nc.dram_tensor("b0", [S0, 1], dt.float32, kind="ExternalInput")
    b1_d = nc.dram_tensor("b1", [S1, 1], dt.float32, kind="ExternalInput")
    out_d = nc.dram_tensor("out", [S0 - NHF + S1, BC], dt.float32,
                           kind="ExternalOutput")

    with tile.TileContext(nc) as tc:
        with (
            tc.tile_pool(name="const", bufs=1) as const_pool,
            tc.tile_pool(name="xt", bufs=1) as xt_pool,
            tc.tile_pool(name="sq", bufs=12) as sq_pool,
            tc.tile_pool(name="u", bufs=3) as u_pool,
            tc.tile_pool(name="d", bufs=2) as d_pool,
            tc.tile_pool(name="outp", bufs=1) as out_pool,
            tc.tile_pool(name="vps", bufs=3, space="PSUM") as vps_pool,
            tc.tile_pool(name="hps", bufs=1, space="PSUM") as h_pool,
        ):
            p0_sb = const_pool.tile([KU, R0P], dt.bfloat16)
            c0_sb = const_pool.tile([128, NC0 * S0], dt.bfloat16)
            p1_sb = const_pool.tile([KU, R1], dt.bfloat16)
            c1p_sb = const_pool.tile([128, NC1 * S1], dt.bfloat16)
            c1s_sb = const_pool.tile([KU, S1], dt.bfloat16)
            b0_sb = const_pool.tile([S0, 1], dt.float32)
            b1_sb = const_pool.tile([S1, 1], dt.float32)
            xt_sb = [xt_pool.tile([KU, XTW], dt.bfloat16, tag=f"xt{t}",
                                  name=f"xt_{t}") for t in range(NXT)]
            nc.gpsimd.dma_start(out=xt_sb[0][0:F0, :], in_=xt_d[0][:])
            nc.gpsimd.dma_start(out=xt_sb[0][NHF:KU, :], in_=xt_d[0][:])
            nc.gpsimd.dma_start(out=p0_sb[:], in_=p0_d[:])
            nc.gpsimd.dma_start(out=b0_sb[:], in_=b0_d[:])
            nc.gpsimd.dma_start(out=b1_sb[:], in_=b1_d[:])
            for t in range(1, NXT):
                nc.gpsimd.dma_start(out=xt_sb[t][0:F0, :], in_=xt_d[t][:])
                nc.gpsimd.dma_start(out=xt_sb[t][NHF:KU, :], in_=xt_d[t][:])
            nc.gpsimd.dma_start(out=c0_sb[:], in_=c0_d[:])
            nc.gpsimd.dma_start(out=p1_sb[:], in_=p1_d[:])
            nc.gpsimd.dma_start(out=c1p_sb[:], in_=c1p_d[:])
            nc.gpsimd.dma_start(out=c1s_sb[:], in_=c1s_d[:])

            zeros_sb = const_pool.tile([128, NFREE], dt.bfloat16)
            nc.vector.memset(zeros_sb[:], 0.0)
            out0_sb = out_pool.tile([S0 - NHF, BC], dt.float32, tag="o0")
            out1_sb = out_pool.tile([S1, BC], dt.float32, tag="o1")

            NH2 = NFREE // NMM   # matmul halves per chunk column block

            # Even spread of chain-squares (DVE cast + GpSimd mul) among the
            # 27 per-chunk PSUM squares; the rest go to ACT directly.
            NSQ = NC0 + NC1
            N_CHAIN = 10
            CHAIN_SET = {m for m in range(NSQ)
                         if (m * N_CHAIN) // NSQ != ((m + 1) * N_CHAIN) // NSQ}

            def square(idx, dst, src):
                # PSUM evacuation split: ACT squares directly; the rest go
                # DVE copy (PSUM->SBUF bf16) + GpSimd multiply (SBUF bf16).
                # DVE cannot read the PSUM operand twice (NCC_IBVF027).
                if idx not in CHAIN_SET:
                    nc.scalar.square(dst, src)
                else:
                    tmp = sq_pool.tile(list(dst.shape), dst.dtype,
                                       tag="sqtmp", bufs=6)
                    nc.vector.tensor_copy(out=tmp[:], in_=src)
                    nc.gpsimd.tensor_mul(dst, tmp[:], tmp[:])

            def xap_of(ch):
                xt = xt_sb[(ch * NFREE) // XTW]
                c0_ = (ch * NFREE) % XTW
                return xt[:, c0_:c0_ + NFREE]

            def mm_s1l0(ch, m, vps, row):
                # row 0: PE rows 0:40; row 1: PE rows 64:104 (tile_position
                # packing -- two K=40 matmuls run concurrently)
                lo, hi = (0, F0) if row == 0 else (NHF, KU)
                xa = xap_of(ch)
                for h in range(NH2):
                    hs = slice(h * NMM, (h + 1) * NMM)
                    mi = nc.tensor.matmul(
                        vps[:, hs], p0_sb[lo:hi, m * 128:(m + 1) * 128],
                        xa[lo:hi, hs], start=True, stop=True,
                        tile_position=(lo, 0),
                    )
                    if h > 0:
                        mi.ins.ldweights = False

            def mm_s3l0(ch, m, h0ps, v0sq):
                for h in range(NH2):
                    hs = slice(h * NMM, (h + 1) * NMM)
                    mi = nc.tensor.matmul(
                        h0ps[:, hs], c0_sb[:, m * S0:(m + 1) * S0],
                        v0sq[m][:, hs], start=(m == 0), stop=(m == NC0 - 1),
                    )
                    if h > 0:
                        mi.ins.ldweights = False

            def mm_s1l1(ch, m, vps, u):
                for h in range(NH2):
                    hs = slice(h * NMM, (h + 1) * NMM)
                    mi = nc.tensor.matmul(
                        vps[:, hs], p1_sb[:, m * 128:(m + 1) * 128],
                        u[:, hs], start=True, stop=True,
                    )
                    if h > 0:
                        mi.ins.ldweights = False

            def mm_s3l1(ch, m, h1ps, v1sq, usq):
                for h in range(NH2):
                    hs = slice(h * NMM, (h + 1) * NMM)
                    if m < NC1:
                        mi = nc.tensor.matmul(
                            h1ps[:, hs], c1p_sb[:, m * S1:(m + 1) * S1],
                            v1sq[m][:, hs], start=(m == 0), stop=False,
                        )
                    else:
                        mi = nc.tensor.matmul(
                            h1ps[:, hs], c1s_sb[:], usq[:, hs],
                            start=False, stop=True,
                        )
                    if h > 0:
                        mi.ins.ldweights = False

            # Per-chunk live state, keyed by chunk index.
            st = {}

            for i in range(NCHUNKS + 2):
                cA = i          # S1L0 of chunk i
                cB = i - 1      # S3L0 + post0 + S1L1 of chunk i-1
                cE = i - 2      # S3L1 + post1 of chunk i-2

                if cA < NCHUNKS:
                    st[cA] = {"v0sq": [], "v1sq": []}

                # ---- step 1: S3L0(cB) interleaved with S1L0(cA) ----
                for m in range(NC0):
                    if cA < NCHUNKS and m % 2 == 0:
                        # emit a packed pair (m, m+1) of K=40 projections
                        for mm in (m, m + 1):
                            if mm >= NC0:
                                break
                            vps = vps_pool.tile([128, NFREE], dt.float32,
                                                tag="vps", name=f"vps0_{cA}_{mm}")
                            mm_s1l0(cA, mm, vps, mm - m)
                            vsq = sq_pool.tile([128, NFREE], dt.bfloat16,
                                               tag="v0sq", bufs=16,
                                               name=f"v0sq_{cA}_{mm}")
                            square(mm, vsq[:], vps[:])
                            st[cA]["v0sq"].append(vsq)
                    if 0 <= cB < NCHUNKS:
                        if m == 0:
                            st[cB]["h0ps"] = h_pool.tile(
                                [S0, NFREE], dt.float32, tag="hps",
                                name=f"h0ps_{cB}")
                        mm_s3l0(cB, m, st[cB]["h0ps"], st[cB]["v0sq"])

                # ---- step 2: post0(cB): relu->u, x copy, d0 relu+reduce ----
                if 0 <= cB < NCHUNKS:
                    h0ps = st[cB]["h0ps"]
                    u = u_pool.tile([KU, NFREE], dt.bfloat16, tag="u")
                    nc.vector.tensor_copy(out=u[NHF:KU, :],
                                          in_=xap_of(cB)[NHF:KU, :])
                    nc.scalar.activation(u[0:NHF, :], h0ps[0:NHF, :], AF.Relu,
                                         bias=b0_sb[0:NHF, 0:1], scale=1.0)
                    d0 = d_pool.tile([S0 - NHF, NBPC, D], dt.float32, tag="d0")
                    nc.scalar.activation(d0[:], h0ps[NHF:S0, :], AF.Relu,
                                         bias=b0_sb[NHF:S0, 0:1], scale=1.0)
                    nc.vector.tensor_reduce(
                        out=out0_sb[:, cB * NBPC:(cB + 1) * NBPC],
                        in_=d0[:], axis=mybir.AxisListType.X, op=ALU.add,
                    )
                    usq = u_pool.tile([KU, NFREE], dt.bfloat16, tag="usq")
                    nc.vector.tensor_mul(usq[:], u[:], u[:])
                    st[cB]["u"] = u
                    st[cB]["usq"] = usq

                # ---- step 3: S3L1(cE) interleaved with S1L1(cB) ----
                e_list = list(range(NC1 + 1)) if 0 <= cE else []
                d_list = list(range(NC1)) if 0 <= cB < NCHUNKS else []
                ei = 0
                if e_list:
                    st[cE]["h1ps"] = h_pool.tile([S1, NFREE], dt.float32,
                                                 tag="hps",
                                                 name=f"h1ps_{cE}")
                    for _ in range(2):
                        if ei < len(e_list):
                            mm_s3l1(cE, e_list[ei], st[cE]["h1ps"],
                                    st[cE]["v1sq"], st[cE]["usq"])
                            ei += 1
                for m in d_list:
                    vps = vps_pool.tile([128, NFREE], dt.float32, tag="vps")
                    mm_s1l1(cB, m, vps, st[cB]["u"])
                    vsq = sq_pool.tile([128, NFREE], dt.bfloat16,
                                       tag="v1sq", bufs=32)
                    square(NC0 + m, vsq[:], vps[:])
                    st[cB]["v1sq"].append(vsq)
                    if e_list and ei < len(e_list):
                        mm_s3l1(cE, e_list[ei], st[cE]["h1ps"],
                                st[cE]["v1sq"], st[cE]["usq"])
                        ei += 1
                while e_list and ei < len(e_list):
                    mm_s3l1(cE, e_list[ei], st[cE]["h1ps"],
                            st[cE]["v1sq"], st[cE]["usq"])
                    ei += 1

                # ---- step 4: post1(cE): relu d1 + reduce (emitted right
                # after the final s3l1 matmul so the shared PSUM slot frees
                # before the next iteration's s3l0) ----
                if 0 <= cE:
                    d1 = d_pool.tile([S1, NBPC, D], dt.float32, tag="d1")
                    nc.scalar.activation(d1[:], st[cE]["h1ps"][:], AF.Relu,
                                         bias=b1_sb[:, 0:1], scale=1.0)
                    nc.vector.tensor_reduce(
                        out=out1_sb[:, cE * NBPC:(cE + 1) * NBPC],
                        in_=d1[:], axis=mybir.AxisListType.X, op=ALU.add,
                    )
                    del st[cE]

            nc.gpsimd.dma_start(out=out_d[0:S0 - NHF, :], in_=out0_sb[:])
            nc.gpsimd.dma_start(out=out_d[S0 - NHF:, :], in_=out1_sb[:])

    _split_multi_waits(nc)
    return nc


def kernel(x, W0, b0, W1, b1):
    from concourse.bass_utils import run_bass_kernel_spmd

    _patch_ldw_opt()

    x = np.asarray(x)
    w = _build_host_weights(W0, b0, W1, b1)

    if "nc" not in _cached:
        _cached["nc"] = _build_nc()
    nc = _cached["nc"]

    NXT = 4
    XTW = BD // NXT
    in_maps = []
    for c in range(NCORES):
        xs = x[c * BC:(c + 1) * BC]                        # [256, 40, 32]
        xtc = np.ascontiguousarray(
            xs.transpose(1, 0, 2).reshape(F0, BD)
        ).astype(BF16)                                     # [40, 8192]
        m = {f"xt{t}": np.ascontiguousarray(xtc[:, t * XTW:(t + 1) * XTW])
             for t in range(NXT)}
        m.update(w)
        in_maps.append(m)

    import os
    trace = bool(os.environ.get("CIN_TRACE"))
    res = run_bass_kernel_spmd(nc, in_maps, list(range(NCORES)), trace=trace)
    _cached["last_res"] = res
    outs = []
    for c in range(NCORES):
        o = res.results[c]["out"]                          # [192, 256]
        outs.append(np.ascontiguousarray(o.T))             # [256, 192]
    return np.concatenate(outs, axis=0).astype(np.float32)

